# revision 1
# baseline (speedup 1.0000x reference)
"""Trainium2 Bass kernel for the scatter_memory GRU memory-update module.

Computation (torch GRUCell semantics, chunk order r, z, n):
    current = memory[node_ids]                       # [B, H] gather
    gi = messages @ W_ih.T + b_ih ; gh = current @ W_hh.T + b_hh
    r = sigmoid(gi_r + gh_r) ; z = sigmoid(gi_z + gh_z)
    n = tanh(gi_n + r * gh_n)
    updated = (1 - z) * n + z * current
    new_memory = memory.at[node_ids].set(updated)    # scatter

Distribution: the B updated rows are sharded contiguously across 8
NeuronCores.  The gather/scatter over the 500k-row table and the
feature-major transposes run on the host; each core runs the GRU math on
its own [H, B/8] shard (feature dim H=128 sits on the SBUF partition
axis, so the GRU biases become per-partition vectors that fuse into the
ScalarEngine activation ops for free).

Per-core engine layout (chosen from CoreSim engine-busy analysis; ACT is
the pacing engine at ~40us, everything else is kept below it):
  PE   gate matmuls (bf16) + an identity-matmul that accumulates
       t = r*(gh_n + b_hn) into p_in's still-open PSUM group — this
       replaces a second PSUM-reading DVE add (DVE was the old bottleneck)
  ACT  sigmoid(r), sigmoid(z) per 1024-tile, tanh straight from PSUM
  DVE  t = (p_hn + b_hn)*r (STT, the only PSUM-touching DVE op) and the
       blend out = n + z*(h-n) as three all-bf16 SBUF TensorTensor ops
       (2x DVE perf mode); GPSIMD cannot access PSUM (BIR verifier)
  Pool tail-chunk blends only (bf16 SBUF), keeping DVE clear for the drain
A one-tile-deep software pipeline defers each tile's identity-matmul and
tanh into the next tile so neither PE nor ACT stalls on the t chain.
"""

import os
import sys

import numpy as np

for _p in ("/opt/trn_rl_repo", "/root/.axon_site/_ro/trn_rl_repo"):
    if os.path.isdir(_p) and _p not in sys.path:
        sys.path.insert(0, _p)

import ml_dtypes
from contextlib import ExitStack

import concourse.bass as bass
import concourse.tile as tile
from concourse import mybir
from concourse.bass_utils import run_bass_kernel_spmd

BF16 = ml_dtypes.bfloat16
import json as _json

N_CORES = 8
H = 128
NTILE = 1024         # batch columns per PSUM tile (2 banks of fp32 per gate)
CHUNK = 1024         # batch columns per DMA chunk / wide elementwise ops

# exposed for test harnesses
LAST_RESULT = None

_NC_CACHE = {}


def _split_sync_waits(bir: dict) -> dict:
    """Hoist extra per-instruction semaphore waits into standalone
    EventSemaphore instructions.

    The walrus build in this container encodes at most ONE sync wait per
    instruction ("Too many sync wait commands" otherwise); Tile attaches
    one wait per dependency.  An engine-level standalone wait immediately
    before the instruction is semantically identical (the engine stalls
    either way), so keep the last wait inline and hoist the rest.
    """
    n = 0
    for fn in bir.get("functions", []):
        for blk in fn.get("blocks", []):
            out = []
            for inst in blk.get("instructions", []):
                si = inst.get("sync_info") or {}
                ow = si.get("on_wait") or []
                if len(ow) > 1:
                    for w in ow[:-1]:
                        n += 1
                        out.append({
                            "debug": inst.get("debug", 0),
                            "engine": inst["engine"],
                            "ins": [],
                            "outs": [],
                            "name": f"hoistw_{n}_{inst['name']}",
                            "opcode": "EventSemaphore",
                            "sync_info": {"on_update": [], "on_wait": [w]},
                        })
                    si["on_wait"] = [ow[-1]]
                out.append(inst)
            blk["instructions"] = out
    return bir


def _patch_json(nc: bass.Bass) -> None:
    orig = nc.to_json_bytes

    def patched() -> bytes:
        return _json.dumps(_split_sync_waits(_json.loads(orig()))).encode()

    nc.to_json_bytes = patched


def _build_nc(bpc: int) -> bass.Bass:
    """Bass program for one core: GRU over a [H, bpc] feature-major shard."""
    assert bpc % 256 == 0 and CHUNK % NTILE == 0
    f32 = mybir.dt.float32
    bf16 = mybir.dt.bfloat16
    sig = mybir.ActivationFunctionType.Sigmoid
    tanh = mybir.ActivationFunctionType.Tanh
    add_op = mybir.AluOpType.add
    mult_op = mybir.AluOpType.mult

    nc = bass.Bass()
    # wT = [w_ih.T | w_hh.T | I] (I closes the p_in accumulation group on
    # PE); xh packs x and h per partition so ONE DMA per chunk brings both
    WCOLS = 7 * H
    wT = nc.declare_dram_parameter("wT", [H, WCOLS], bf16, isOutput=False)
    xh = nc.declare_dram_parameter("xh", [H, 2, bpc], bf16, isOutput=False)
    # bias columns: 0 = b_ih_r + b_hh_r, 1 = b_ih_z + b_hh_z, 2 = b_hh_n, 3 = b_ih_n
    biases = nc.declare_dram_parameter("biases", [H, 4], f32, isOutput=False)
    outT = nc.declare_dram_parameter("outT", [H, bpc], bf16, isOutput=True)

    with ExitStack() as ctx:
        tc = ctx.enter_context(tile.TileContext(nc))
        singles = ctx.enter_context(tc.tile_pool(name="singles", bufs=1))
        # enough io buffers that every chunk's x/h DMA can be issued up
        # front — the DMA queues stream while compute follows behind
        io = ctx.enter_context(tc.tile_pool(name="io", bufs=8))
        wide = ctx.enter_context(tc.tile_pool(name="wide", bufs=2))
        mids = ctx.enter_context(tc.tile_pool(name="mids", bufs=4))
        # 4 PSUM tensors x [128, 1024] fp32 = 2 banks each -> all 8 banks
        psum = ctx.enter_context(tc.tile_pool(name="psum", bufs=1, space="PSUM"))

        # dummy sigmoid on a freshly-memset tile fires the ~1.4us ACT table
        # load immediately (zero DMA dependencies), so it overlaps the DMA
        # ramp instead of stalling the first real sigmoid
        warm_in = singles.tile([H, 1], f32)
        nc.vector.memset(warm_in, 0.0)
        warm_sb = singles.tile([H, 1], f32)
        nc.scalar.activation(out=warm_sb, in_=warm_in,
                             func=sig, bias=0.0, scale=1.0)

        # weights go over SWDGE (issues in parallel with SP's xh0 HWDGE
        # issue; the DMA device serializes transfers either way but the
        # issue+DGE latencies overlap), then bias
        wx0_sb = singles.tile([H, WCOLS], bf16)
        nc.gpsimd.dma_start(out=wx0_sb, in_=wT[:, :])
        b_sb = singles.tile([H, 4], f32)
        nc.gpsimd.dma_start(out=b_sb, in_=biases[:, :])
        eye_sb = wx0_sb[:, 6 * H : 7 * H]

        # small first chunks so compute starts before the first big DMA
        # lands; tapered last chunks so the final serial chain is short
        tail_plan = (1024, 1024, 768, 512, 256)
        chunks = []
        pos = 0
        for head in (512, 1024):
            if pos < bpc:
                csz = min(head, bpc - pos)
                chunks.append((pos, csz))
                pos += csz
        while bpc - pos > sum(tail_plan):
            chunks.append((pos, CHUNK))
            pos += CHUNK
        for tail in tail_plan:
            if pos >= bpc:
                break
            csz = min(tail, bpc - pos)
            chunks.append((pos, csz))
            pos += csz
        assert pos == bpc, (pos, bpc, chunks)
        # keep the drain chunk small: its chain is serial end-to-end
        while chunks[-1][1] > 512:
            c0l, cszl = chunks.pop()
            chunks.append((c0l, cszl - 256))
            chunks.append((c0l + cszl - 256, 256))

        assert chunks[0][1] == 512

        # Software pipeline, one tile deep: tile i's `p_in += I @ t_i`
        # (closing the open accumulation group) is emitted inside tile i+1's
        # matmul block, and tile i's tanh after tile i+1's sigmoids — so
        # neither PE nor ACT ever stalls waiting for t_i.
        pending_imm = []   # callbacks emitting the deferred identity-matmul
        pending_tanh = []  # (emit_tanh, blend_or_None) callbacks

        def pop_tanh():
            if pending_tanh:
                emit, blend = pending_tanh.pop(0)
                emit()
                if blend is not None:
                    blend()

        for ci, (c0, csz) in enumerate(chunks):
            xh_t = io.tile([H, 2 * csz], bf16, tag="x")
            nc.sync.dma_start(out=xh_t, in_=xh[:, :, c0 : c0 + csz])
            h_sb = xh_t[:, csz : 2 * csz]
            def xsl(s, _t=xh_t):
                return _t[:, s]
            o_sb = wide.tile([H, csz], bf16, tag="o", bufs=2)
            n_ch = wide.tile([H, csz], bf16, tag="n", bufs=2)
            z_ch = wide.tile([H, csz], bf16, tag="z", bufs=2)
            last = ci == len(chunks) - 1
            if last:
                # complete all deferred work before the inline drain chunk
                while pending_imm:
                    pending_imm.pop(0)()
                while pending_tanh:
                    pop_tanh()

            ntiles = -(-csz // NTILE)
            for tix, t0 in enumerate(range(0, csz, NTILE)):
                tsz = min(NTILE, csz - t0)
                sl = slice(t0, t0 + tsz)
                p_r = psum.tile([H, tsz], f32, tag="p_r")
                p_z = psum.tile([H, tsz], f32, tag="p_z")
                p_in = psum.tile([H, tsz], f32, tag="p_in")
                p_hn = psum.tile([H, tsz], f32, tag="p_hn")

                # gate pre-activations, 512 fp32 per matmul (one PSUM bank).
                # p_in's accumulation group stays OPEN (stop=False).
                # previous tile's deferred I-matmul lands FIRST: it must
                # precede this tile's p_in matmul in PE order (that matmul
                # WAR-waits on the previous tanh, which needs the I-matmul —
                # emitting it later would deadlock the PE queue)
                if pending_imm:
                    pending_imm.pop(0)()
                for q0 in range(0, tsz, 512):
                    qsz = min(512, tsz - q0)
                    qs = slice(t0 + q0, t0 + q0 + qsz)
                    qd = slice(q0, q0 + qsz)
                    nc.tensor.matmul(p_r[:, qd], wx0_sb[:, 0:H], xsl(qs),
                                     start=True, stop=False)
                    nc.tensor.matmul(p_r[:, qd], wx0_sb[:, 3 * H : 4 * H],
                                     h_sb[:, qs], start=False, stop=True)
                    nc.tensor.matmul(p_z[:, qd], wx0_sb[:, H : 2 * H],
                                     xsl(qs), start=True, stop=False)
                    nc.tensor.matmul(p_z[:, qd], wx0_sb[:, 4 * H : 5 * H],
                                     h_sb[:, qs], start=False, stop=True)
                    nc.tensor.matmul(p_in[:, qd], wx0_sb[:, 2 * H : 3 * H],
                                     xsl(qs), start=True, stop=last)
                    nc.tensor.matmul(p_hn[:, qd], wx0_sb[:, 5 * H : 6 * H],
                                     h_sb[:, qs], start=True, stop=True)

                r_t = mids.tile([H, tsz], bf16, tag="r")
                nc.scalar.activation(out=r_t, in_=p_r, func=sig,
                                     bias=b_sb[:, 0:1], scale=1.0)
                nc.scalar.activation(out=z_ch[:, sl], in_=p_z, func=sig,
                                     bias=b_sb[:, 1:2], scale=1.0)

                if last:
                    # drain chunk: no deferral, no PE in the critical chain.
                    # zh = z*h precomputed on GpSimd; DVE does t, pre, v, o
                    # back-to-back; the out DMA goes via the (now idle) ACT
                    # HWDGE queue so it isn't stuck behind SP's in-order queue
                    zh_ch = wide.tile([H, csz], bf16, tag="w", bufs=2)
                    nc.gpsimd.tensor_mul(out=zh_ch, in0=z_ch, in1=h_sb)
                    t_t = mids.tile([H, tsz], bf16, tag="t")
                    nc.vector.scalar_tensor_tensor(
                        out=t_t, in0=p_hn, scalar=b_sb[:, 2:3], in1=r_t,
                        op0=add_op, op1=mult_op)
                    pre_t = mids.tile([H, tsz], bf16, tag="pre")
                    nc.vector.tensor_add(out=pre_t, in0=t_t, in1=p_in)
                    nc.scalar.activation(out=n_ch[:, sl], in_=pre_t,
                                         func=tanh, bias=b_sb[:, 3:4],
                                         scale=1.0)
                    v_ch = wide.tile([H, csz], bf16, tag="y", bufs=2)
                    nc.vector.scalar_tensor_tensor(
                        out=v_ch, in0=z_ch, scalar=1.0, in1=n_ch,
                        op0=mybir.AluOpType.subtract, op1=mult_op)
                    nc.vector.tensor_sub(out=o_sb, in0=zh_ch, in1=v_ch)
                    # final DMA on the (now idle) ACT queue: SP's in-order
                    # queue still holds the previous chunk's out-DMA
                    nc.scalar.dma_start(out=outT[:, c0 : c0 + csz], in_=o_sb)
                    continue

                # previous tile's tanh: its I-matmul is several PE slots back
                pop_tanh()

                # t = (h_n + b_hn) * r on DVE (GPSIMD cannot touch PSUM)
                t_t = mids.tile([H, tsz], bf16, tag="t")
                nc.vector.scalar_tensor_tensor(
                    out=t_t, in0=p_hn, scalar=b_sb[:, 2:3], in1=r_t,
                    op0=add_op, op1=mult_op)

                def imm(_p=p_in, _t=t_t, _tsz=tsz):
                    for q0 in range(0, _tsz, 512):
                        qsz = min(512, _tsz - q0)
                        qd = slice(q0, q0 + qsz)
                        nc.tensor.matmul(_p[:, qd], eye_sb, _t[:, qd],
                                         start=False, stop=True)
                pending_imm.append(imm)

                def emit_tanh(_p=p_in, _n=n_ch, _sl=sl):
                    nc.scalar.activation(out=_n[:, _sl], in_=_p, func=tanh,
                                         bias=b_sb[:, 3:4], scale=1.0)

                blend = None
                if tix == ntiles - 1:
                    # out = n + z*(h-n): three all-bf16 SBUF TensorTensor
                    # ops, in 2x perf mode on DVE. The two big late chunks'
                    # blends go to the idle Pool engine (keeps DVE clear for
                    # the drain-critical t ops) — but NOT the second-to-last
                    # chunk: Pool's serial backlog would delay its out-DMA,
                    # and the final chunk's DMA queues behind it on SP.
                    on_pool = len(chunks) - 5 <= ci < len(chunks) - 2
                    eng = nc.gpsimd if on_pool else nc.vector

                    def blend(_z=z_ch, _n=n_ch, _h=h_sb, _o=o_sb,
                              _c0=c0, _csz=csz, _e=eng):
                        w_ch = wide.tile([H, _csz], bf16, tag="w", bufs=2)
                        y_ch = wide.tile([H, _csz], bf16, tag="y", bufs=2)
                        _e.tensor_sub(out=w_ch, in0=_h, in1=_n)
                        _e.tensor_mul(out=y_ch, in0=_z, in1=w_ch)
                        _e.tensor_add(out=_o, in0=y_ch, in1=_n)
                        nc.sync.dma_start(out=outT[:, _c0 : _c0 + _csz], in_=_o)
                pending_tanh.append((emit_tanh, blend))

        # drain the one-deep pipeline (last chunk was handled inline)
        while pending_imm:
            pending_imm.pop(0)()
        while pending_tanh:
            pop_tanh()

    _patch_json(nc)
    return nc


def _get_nc(bpc: int) -> bass.Bass:
    if bpc not in _NC_CACHE:
        _NC_CACHE[bpc] = _build_nc(bpc)
    return _NC_CACHE[bpc]


def kernel(node_ids, messages, memory, W_ih, W_hh, b_ih, b_hh):
    global LAST_RESULT
    node_ids = np.asarray(node_ids)
    messages = np.asarray(messages, dtype=np.float32)
    memory = np.asarray(memory, dtype=np.float32)
    W_ih = np.asarray(W_ih, dtype=np.float32)
    W_hh = np.asarray(W_hh, dtype=np.float32)
    b_ih = np.asarray(b_ih, dtype=np.float32)
    b_hh = np.asarray(b_hh, dtype=np.float32)

    B = node_ids.shape[0]
    per = -(-B // N_CORES)                       # rows per core (unpadded)
    bpc = -(-per // 256) * 256                   # padded to 256 multiple
    nc = _get_nc(bpc)

    current = memory[node_ids]                   # [B, H] host gather

    w_ihT = np.ascontiguousarray(W_ih.T).astype(BF16)
    w_hhT = np.ascontiguousarray(W_hh.T).astype(BF16)
    bias = np.empty((H, 4), dtype=np.float32)
    bias[:, 0] = b_ih[0:H] + b_hh[0:H]
    bias[:, 1] = b_ih[H : 2 * H] + b_hh[H : 2 * H]
    bias[:, 2] = b_hh[2 * H : 3 * H]
    bias[:, 3] = b_ih[2 * H : 3 * H]

    wT = np.zeros((H, 7 * H), dtype=BF16)
    wT[:, 0 : 3 * H] = w_ihT
    wT[:, 3 * H : 6 * H] = w_hhT
    wT[:, 6 * H : 7 * H] = np.eye(H, dtype=BF16)
    in_maps = []
    for c in range(N_CORES):
        lo = c * per
        hi = min(lo + per, B)
        xh = np.zeros((H, 2, bpc), dtype=BF16)
        if hi > lo:
            xh[:, 0, : hi - lo] = messages[lo:hi].T
            xh[:, 1, : hi - lo] = current[lo:hi].T
        in_maps.append({"wT": wT, "xh": xh, "biases": bias})

    res = run_bass_kernel_spmd(nc, in_maps, list(range(N_CORES)))
    LAST_RESULT = res

    updated = np.empty((B, H), dtype=np.float32)
    for c in range(N_CORES):
        lo = c * per
        hi = min(lo + per, B)
        if hi > lo:
            updated[lo:hi] = res.results[c]["outT"][:, : hi - lo].T.astype(np.float32)

    new_memory = memory.copy()
    new_memory[node_ids] = updated
    return new_memory



# revision 2
# speedup vs baseline: 1.0393x; 1.0393x over previous
"""Trainium2 Bass kernel for the scatter_memory GRU memory-update module.

Computation (torch GRUCell semantics, chunk order r, z, n):
    current = memory[node_ids]                       # [B, H] gather
    gi = messages @ W_ih.T + b_ih ; gh = current @ W_hh.T + b_hh
    r = sigmoid(gi_r + gh_r) ; z = sigmoid(gi_z + gh_z)
    n = tanh(gi_n + r * gh_n)
    updated = (1 - z) * n + z * current
    new_memory = memory.at[node_ids].set(updated)    # scatter

Distribution: the B updated rows are sharded contiguously across 8
NeuronCores.  The gather/scatter over the 500k-row table and the
feature-major transposes run on the host; each core runs the GRU math on
its own [H, B/8] shard (feature dim H=128 sits on the SBUF partition
axis, so the GRU biases become per-partition vectors that fuse into the
ScalarEngine activation ops for free).

Per-core engine layout (from the HW NTFF profile of the previous
revision; ACT is the pacing engine at ~1.15us per 1024-column tile,
3 transcendentals per element is inherent to the GRU):
  PE   pure feed-forward gate matmuls (bf16, weight-outer order) — no
       identity-matmul accumulate, so PE never waits on DVE/ACT results
       and streams at full p-state
  ACT  sigmoid(r), sigmoid(z) per chunk, plus the PREVIOUS chunk's tanh
       (one-chunk software pipeline so ACT never waits on the DVE chain)
  DVE  t = (p_hn + b_hn)*r and pre = p_in + t (the two PSUM-touching
       ops), plus the blend out = n + z*(h-n) for even chunks
  Pool blend for odd chunks (whole 3-op blend, spread across the run —
       the previous revision sent late-chunk blends to Pool which
       serialized into a ~15us tail)
DMA order on the Sync HWDGE queue: weights + biases FIRST (the previous
revision queued 5 big xh chunks ahead of the weights, pushing the first
matmul to 12.5us), then tapered xh chunks (256/512 head, 512/256 tail)
interleaved with output writebacks.
"""

import os
import sys

import numpy as np

for _p in ("/opt/trn_rl_repo", "/root/.axon_site/_ro/trn_rl_repo"):
    if os.path.isdir(_p) and _p not in sys.path:
        sys.path.insert(0, _p)

import ml_dtypes
from contextlib import ExitStack

import concourse.bass as bass
import concourse.tile as tile
from concourse import mybir
from concourse.bass_utils import run_bass_kernel_spmd

BF16 = ml_dtypes.bfloat16
import json as _json

N_CORES = 8
H = 128

# exposed for test harnesses
LAST_RESULT = None

_NC_CACHE = {}


def _split_sync_waits(bir: dict) -> dict:
    """Hoist extra per-instruction semaphore waits into standalone
    EventSemaphore instructions.

    The walrus build in this container encodes at most ONE sync wait per
    instruction ("Too many sync wait commands" otherwise); Tile attaches
    one wait per dependency.  An engine-level standalone wait immediately
    before the instruction is semantically identical (the engine stalls
    either way), so keep the last wait inline and hoist the rest.
    """
    n = 0
    for fn in bir.get("functions", []):
        for blk in fn.get("blocks", []):
            out = []
            for inst in blk.get("instructions", []):
                si = inst.get("sync_info") or {}
                ow = si.get("on_wait") or []
                if len(ow) > 1:
                    for w in ow[:-1]:
                        n += 1
                        out.append({
                            "debug": inst.get("debug", 0),
                            "engine": inst["engine"],
                            "ins": [],
                            "outs": [],
                            "name": f"hoistw_{n}_{inst['name']}",
                            "opcode": "EventSemaphore",
                            "sync_info": {"on_update": [], "on_wait": [w]},
                        })
                    si["on_wait"] = [ow[-1]]
                out.append(inst)
            blk["instructions"] = out
    return bir


def _patch_json(nc: bass.Bass) -> None:
    orig = nc.to_json_bytes

    def patched() -> bytes:
        return _json.dumps(_split_sync_waits(_json.loads(orig()))).encode()

    nc.to_json_bytes = patched


def _chunk_plan(bpc: int) -> list[tuple[int, int]]:
    """Tapered chunks: small head so compute starts as soon as the first
    DMA lands, small tail so the final serial chain is short."""
    head = [256, 512, 768]
    tail = [512, 256]
    mid = bpc - sum(head) - sum(tail)
    assert mid >= 0 and mid % 256 == 0, (bpc, mid)
    chunks_sz = list(head)
    while mid > 1024:
        chunks_sz.append(1024)
        mid -= 1024
    if mid:
        chunks_sz.append(mid)
    chunks_sz += tail
    out = []
    pos = 0
    for c in chunks_sz:
        out.append((pos, c))
        pos += c
    assert pos == bpc, (pos, bpc)
    return out


def _build_nc(bpc: int) -> bass.Bass:
    """Bass program for one core: GRU over a [H, bpc] feature-major shard."""
    assert bpc % 256 == 0
    f32 = mybir.dt.float32
    bf16 = mybir.dt.bfloat16
    sig = mybir.ActivationFunctionType.Sigmoid
    tanh = mybir.ActivationFunctionType.Tanh
    add_op = mybir.AluOpType.add
    mult_op = mybir.AluOpType.mult

    nc = bass.Bass()
    # wT column blocks, weight-outer order: r_x, r_h, z_x, z_h, n_x, n_h
    WCOLS = 6 * H
    wT = nc.declare_dram_parameter("wT", [H, WCOLS], bf16, isOutput=False)
    # xh packs x and h per partition so ONE DMA per chunk brings both
    xh = nc.declare_dram_parameter("xh", [H, 2, bpc], bf16, isOutput=False)
    # bias columns: 0 = b_ih_r + b_hh_r, 1 = b_ih_z + b_hh_z, 2 = b_hh_n, 3 = b_ih_n
    biases = nc.declare_dram_parameter("biases", [H, 4], f32, isOutput=False)
    outT = nc.declare_dram_parameter("outT", [H, bpc], bf16, isOutput=True)

    chunks = _chunk_plan(bpc)
    PREFETCH = 4  # xh chunk DMAs in flight ahead of compute

    with ExitStack() as ctx:
        tc = ctx.enter_context(tile.TileContext(nc))
        singles = ctx.enter_context(tc.tile_pool(name="singles", bufs=1))
        io = ctx.enter_context(tc.tile_pool(name="io", bufs=PREFETCH + 2))
        zs = ctx.enter_context(tc.tile_pool(name="zs", bufs=4))
        mids = ctx.enter_context(tc.tile_pool(name="mids", bufs=3))
        wide = ctx.enter_context(tc.tile_pool(name="wide", bufs=3))
        # 4 PSUM tensors x [128, 1024] fp32 = 2 banks each -> all 8 banks
        psum = ctx.enter_context(tc.tile_pool(name="psum", bufs=1, space="PSUM"))

        # dummy sigmoid on a freshly-memset tile fires the ~1.3us ACT table
        # load immediately (zero DMA dependencies), so it overlaps the DMA
        # ramp instead of stalling the first real sigmoid
        warm_in = singles.tile([H, 1], f32)
        nc.vector.memset(warm_in, 0.0)
        warm_sb = singles.tile([H, 1], f32)
        nc.scalar.activation(out=warm_sb, in_=warm_in,
                             func=sig, bias=0.0, scale=1.0)

        # weights + biases go FIRST on the Sync HWDGE queue: everything
        # else waits on them, and the queue is drained in FIFO order
        wx0_sb = singles.tile([H, WCOLS], bf16)
        nc.sync.dma_start(out=wx0_sb, in_=wT[:, :])
        b_sb = singles.tile([H, 4], f32)
        nc.sync.dma_start(out=b_sb, in_=biases[:, :])

        def issue_xh(ci: int):
            c0, csz = chunks[ci]
            t = io.tile([H, 2 * csz], bf16, tag="xh")
            nc.sync.dma_start(out=t, in_=xh[:, :, c0 : c0 + csz])
            return t

        xh_tiles: dict[int, object] = {}
        for ci in range(min(PREFETCH, len(chunks))):
            xh_tiles[ci] = issue_xh(ci)

        # tail-of-previous-chunk callbacks: emits tanh(c-1) on ACT, the
        # blend for c-1 on DVE (even) / Pool (odd), and the out DMA
        pending = []

        n_ch = len(chunks)
        for ci, (c0, csz) in enumerate(chunks):
            if ci + PREFETCH < n_ch:
                xh_tiles[ci + PREFETCH] = issue_xh(ci + PREFETCH)
            xh_t = xh_tiles.pop(ci)
            x_sb = xh_t[:, 0:csz]
            h_sb = xh_t[:, csz : 2 * csz]

            p_r = psum.tile([H, csz], f32, tag="p_r")
            p_z = psum.tile([H, csz], f32, tag="p_z")
            p_in = psum.tile([H, csz], f32, tag="p_in")
            p_hn = psum.tile([H, csz], f32, tag="p_hn")

            # PE: weight-outer gate matmuls.  Group order r, z, hn, in
            # matches when each PSUM tag is freed by its consumer in the
            # previous cycle (sig_r early, sig_z next, t mid, pre late).
            qs = [(q0, min(512, csz - q0)) for q0 in range(0, csz, 512)]
            for wcol, ptile, rhs, start, stop in (
                (0, p_r, x_sb, True, False),
                (1, p_r, h_sb, False, True),
                (2, p_z, x_sb, True, False),
                (3, p_z, h_sb, False, True),
                (5, p_hn, h_sb, True, True),
                (4, p_in, x_sb, True, True),
            ):
                w_sl = wx0_sb[:, wcol * H : (wcol + 1) * H]
                for q0, qsz in qs:
                    qd = slice(q0, q0 + qsz)
                    nc.tensor.matmul(ptile[:, qd], w_sl, rhs[:, qd],
                                     start=start, stop=stop)

            # ACT: this chunk's sigmoids
            r_t = mids.tile([H, csz], bf16, tag="r")
            nc.scalar.activation(out=r_t, in_=p_r, func=sig,
                                 bias=b_sb[:, 0:1], scale=1.0)
            z_t = zs.tile([H, csz], bf16, tag="z")
            nc.scalar.activation(out=z_t, in_=p_z, func=sig,
                                 bias=b_sb[:, 1:2], scale=1.0)

            # previous chunk's tanh + blend + writeback (inputs all ready,
            # so ACT/DVE/Pool never stall on the intra-chunk chain)
            if pending:
                pending.pop(0)()

            # DVE: t = (p_hn + b_hn) * r, then pre = p_in + t
            t_t = mids.tile([H, csz], bf16, tag="t")
            nc.vector.scalar_tensor_tensor(
                out=t_t, in0=p_hn, scalar=b_sb[:, 2:3], in1=r_t,
                op0=add_op, op1=mult_op)
            pre_t = mids.tile([H, csz], bf16, tag="pre")
            nc.vector.scalar_tensor_tensor(
                out=pre_t, in0=p_in, scalar=1.0, in1=t_t,
                op0=mult_op, op1=add_op)

            # blends alternate whole chunks between DVE and Pool so both
            # engines stay under the ACT pace throughout the run; the
            # last two (small) chunks stay on DVE — Pool's ~2us/op
            # latency would stretch the drain
            on_pool = (ci % 2 == 1) and ci < n_ch - 2
            eng = nc.gpsimd if on_pool else nc.vector

            def tail(_pre=pre_t, _z=z_t, _h=h_sb, _c0=c0, _csz=csz, _e=eng):
                nn = zs.tile([H, _csz], bf16, tag="n")
                nc.scalar.activation(out=nn, in_=_pre, func=tanh,
                                     bias=b_sb[:, 3:4], scale=1.0)
                w_ch = wide.tile([H, _csz], bf16, tag="w")
                y_ch = wide.tile([H, _csz], bf16, tag="y")
                o_ch = wide.tile([H, _csz], bf16, tag="o")
                _e.tensor_sub(out=w_ch, in0=_h, in1=nn)
                _e.tensor_mul(out=y_ch, in0=_z, in1=w_ch)
                _e.tensor_add(out=o_ch, in0=y_ch, in1=nn)
                nc.sync.dma_start(out=outT[:, _c0 : _c0 + _csz], in_=o_ch)

            pending.append(tail)

        while pending:
            pending.pop(0)()

    _patch_json(nc)
    return nc


def _get_nc(bpc: int) -> bass.Bass:
    if bpc not in _NC_CACHE:
        _NC_CACHE[bpc] = _build_nc(bpc)
    return _NC_CACHE[bpc]


def kernel(node_ids, messages, memory, W_ih, W_hh, b_ih, b_hh):
    global LAST_RESULT
    node_ids = np.asarray(node_ids)
    messages = np.asarray(messages, dtype=np.float32)
    memory = np.asarray(memory, dtype=np.float32)
    W_ih = np.asarray(W_ih, dtype=np.float32)
    W_hh = np.asarray(W_hh, dtype=np.float32)
    b_ih = np.asarray(b_ih, dtype=np.float32)
    b_hh = np.asarray(b_hh, dtype=np.float32)

    B = node_ids.shape[0]
    per = -(-B // N_CORES)                       # rows per core (unpadded)
    bpc = -(-per // 256) * 256                   # padded to 256 multiple
    nc = _get_nc(bpc)

    current = memory[node_ids]                   # [B, H] host gather

    bias = np.empty((H, 4), dtype=np.float32)
    bias[:, 0] = b_ih[0:H] + b_hh[0:H]
    bias[:, 1] = b_ih[H : 2 * H] + b_hh[H : 2 * H]
    bias[:, 2] = b_hh[2 * H : 3 * H]
    bias[:, 3] = b_ih[2 * H : 3 * H]

    # weight-outer column order: r_x, r_h, z_x, z_h, n_x, n_h
    wT = np.empty((H, 6 * H), dtype=BF16)
    for g in range(3):
        wT[:, (2 * g) * H : (2 * g + 1) * H] = W_ih[g * H : (g + 1) * H].T
        wT[:, (2 * g + 1) * H : (2 * g + 2) * H] = W_hh[g * H : (g + 1) * H].T

    in_maps = []
    for c in range(N_CORES):
        lo = c * per
        hi = min(lo + per, B)
        xh = np.zeros((H, 2, bpc), dtype=BF16)
        if hi > lo:
            xh[:, 0, : hi - lo] = messages[lo:hi].T
            xh[:, 1, : hi - lo] = current[lo:hi].T
        in_maps.append({"wT": wT, "xh": xh, "biases": bias})

    res = run_bass_kernel_spmd(nc, in_maps, list(range(N_CORES)))
    LAST_RESULT = res

    updated = np.empty((B, H), dtype=np.float32)
    for c in range(N_CORES):
        lo = c * per
        hi = min(lo + per, B)
        if hi > lo:
            updated[lo:hi] = res.results[c]["outT"][:, : hi - lo].T.astype(np.float32)

    new_memory = memory.copy()
    new_memory[node_ids] = updated
    return new_memory


# revision 3
# speedup vs baseline: 1.2719x; 1.2238x over previous
"""Trainium2 Bass kernel for the scatter_memory GRU memory-update module.

Computation (torch GRUCell semantics, chunk order r, z, n):
    current = memory[node_ids]                       # [B, H] gather
    gi = messages @ W_ih.T + b_ih ; gh = current @ W_hh.T + b_hh
    r = sigmoid(gi_r + gh_r) ; z = sigmoid(gi_z + gh_z)
    n = tanh(gi_n + r * gh_n)
    updated = (1 - z) * n + z * current
    new_memory = memory.at[node_ids].set(updated)    # scatter

Distribution: the B updated rows are sharded contiguously across 8
NeuronCores.  The gather/scatter over the 500k-row table and the
feature-major transposes run on the host; each core runs the GRU math on
its own [H, B/8] shard (feature dim H=128 sits on the SBUF partition
axis, so the GRU biases become per-partition vectors that fuse into the
ScalarEngine activation ops for free).

Engine assignment (from HW NTFF profiles of two prior revisions):
  PE   6 gate matmuls (bf16) + an identity-matmul that accumulates
       t = r*(gh_n + b_hn) into p_in's still-open PSUM group, deferred
       by one chunk so PE never waits on the ACT->DVE chain
  ACT  sigmoid(r), sigmoid(z) per chunk + the previous chunk's tanh
       straight from PSUM (~1.15us per 1024-tile; ACT is near the pace)
  DVE  t = (p_hn + b_hn)*r (the one PSUM-touching op) and the whole
       blend out = n + z*(h-n) as three bf16 SBUF TensorTensor ops in
       2x perf mode
  Pool UNUSED on purpose: HW profiling showed concurrent GpSimd
       TensorTensor traffic slows DVE's 2x-mode ops 2-3.5x (SBUF port
       contention), a strictly bad trade
DMA order on the Sync HWDGE queue: weights + biases FIRST (queueing big
xh chunks ahead of them delays the first matmul by ~4us), then tapered
xh chunks interleaved with per-chunk output writebacks.
"""

import os
import sys

import numpy as np

for _p in ("/opt/trn_rl_repo", "/root/.axon_site/_ro/trn_rl_repo"):
    if os.path.isdir(_p) and _p not in sys.path:
        sys.path.insert(0, _p)

import ml_dtypes
from contextlib import ExitStack

import concourse.bass as bass
import concourse.tile as tile
from concourse import mybir
from concourse.bass_utils import run_bass_kernel_spmd

BF16 = ml_dtypes.bfloat16
import json as _json

N_CORES = 8
H = 128

# exposed for test harnesses
LAST_RESULT = None

_NC_CACHE = {}


def _split_sync_waits(bir: dict) -> dict:
    """Hoist extra per-instruction semaphore waits into standalone
    EventSemaphore instructions.

    The walrus build in this container encodes at most ONE sync wait per
    instruction ("Too many sync wait commands" otherwise); Tile attaches
    one wait per dependency.  An engine-level standalone wait immediately
    before the instruction is semantically identical (the engine stalls
    either way), so keep the last wait inline and hoist the rest.
    """
    n = 0
    for fn in bir.get("functions", []):
        for blk in fn.get("blocks", []):
            out = []
            for inst in blk.get("instructions", []):
                si = inst.get("sync_info") or {}
                ow = si.get("on_wait") or []
                if len(ow) > 1:
                    for w in ow[:-1]:
                        n += 1
                        out.append({
                            "debug": inst.get("debug", 0),
                            "engine": inst["engine"],
                            "ins": [],
                            "outs": [],
                            "name": f"hoistw_{n}_{inst['name']}",
                            "opcode": "EventSemaphore",
                            "sync_info": {"on_update": [], "on_wait": [w]},
                        })
                    si["on_wait"] = [ow[-1]]
                out.append(inst)
            blk["instructions"] = out
    return bir


def _patch_json(nc: bass.Bass) -> None:
    orig = nc.to_json_bytes

    def patched() -> bytes:
        return _json.dumps(_split_sync_waits(_json.loads(orig()))).encode()

    nc.to_json_bytes = patched


def _chunk_plan(bpc: int) -> list[tuple[int, int]]:
    """Max-size middle chunks (fewer chunks = less per-ACTIVATE fixed
    overhead), small tail so the final serial chain is short."""
    tail = [512, 256]
    mid = bpc - sum(tail)
    sizes = []
    while mid % 1024:
        sizes.append(512)
        mid -= 512
    sizes += [1024] * (mid // 1024)
    sizes += tail
    out = []
    pos = 0
    for c in sizes:
        out.append((pos, c))
        pos += c
    assert pos == bpc, (pos, bpc)
    return out


def _build_nc(bpc: int) -> bass.Bass:
    """Bass program for one core: GRU over a [H, bpc] feature-major shard."""
    assert bpc % 256 == 0
    f32 = mybir.dt.float32
    bf16 = mybir.dt.bfloat16
    sig = mybir.ActivationFunctionType.Sigmoid
    tanh = mybir.ActivationFunctionType.Tanh
    add_op = mybir.AluOpType.add
    mult_op = mybir.AluOpType.mult

    nc = bass.Bass()
    # wT column blocks: r_x, r_h, z_x, z_h, n_x, n_h, I (identity closes
    # the p_in accumulation group on PE)
    WCOLS = 7 * H
    wT = nc.declare_dram_parameter("wT", [H, WCOLS], bf16, isOutput=False)
    # xh packs x and h per partition so ONE DMA per chunk brings both
    xh = nc.declare_dram_parameter("xh", [H, 2, bpc], bf16, isOutput=False)
    # bias columns: 0 = b_ih_r + b_hh_r, 1 = b_ih_z + b_hh_z, 2 = b_hh_n, 3 = b_ih_n
    biases = nc.declare_dram_parameter("biases", [H, 4], f32, isOutput=False)
    outT = nc.declare_dram_parameter("outT", [H, bpc], bf16, isOutput=True)

    chunks = _chunk_plan(bpc)
    PREFETCH = 4  # xh chunk DMAs in flight ahead of compute

    with ExitStack() as ctx:
        tc = ctx.enter_context(tile.TileContext(nc))
        singles = ctx.enter_context(tc.tile_pool(name="singles", bufs=1))
        io = ctx.enter_context(tc.tile_pool(name="io", bufs=PREFETCH + 2))
        zs = ctx.enter_context(tc.tile_pool(name="zs", bufs=4))
        mids = ctx.enter_context(tc.tile_pool(name="mids", bufs=3))
        wide = ctx.enter_context(tc.tile_pool(name="wide", bufs=3))
        # 4 PSUM tensors x [128, 1024] fp32 = 2 banks each -> all 8 banks
        psum = ctx.enter_context(tc.tile_pool(name="psum", bufs=1, space="PSUM"))

        # dummy sigmoid on a freshly-memset tile fires the ~1.3us ACT table
        # load immediately (zero DMA dependencies), so it overlaps the DMA
        # ramp instead of stalling the first real sigmoid
        warm_in = singles.tile([H, 1], f32)
        nc.vector.memset(warm_in, 0.0)
        warm_sb = singles.tile([H, 1], f32)
        nc.scalar.activation(out=warm_sb, in_=warm_in,
                             func=sig, bias=0.0, scale=1.0)

        # weights + biases go FIRST on the Sync HWDGE queue: everything
        # else waits on them, and the queue is drained in FIFO order
        wx0_sb = singles.tile([H, WCOLS], bf16)
        nc.sync.dma_start(out=wx0_sb, in_=wT[:, :])
        b_sb = singles.tile([H, 4], f32)
        nc.sync.dma_start(out=b_sb, in_=biases[:, :])
        eye_sb = wx0_sb[:, 6 * H : 7 * H]

        def issue_xh(ci: int):
            c0, csz = chunks[ci]
            t = io.tile([H, 2 * csz], bf16, tag="xh")
            nc.sync.dma_start(out=t, in_=xh[:, :, c0 : c0 + csz])
            return t

        xh_tiles: dict[int, object] = {}
        for ci in range(min(PREFETCH, len(chunks))):
            xh_tiles[ci] = issue_xh(ci)

        # one-chunk software pipeline: chunk c's identity-matmul (closing
        # p_in's open group with + I @ t) and everything downstream of it
        # (tanh, blend, writeback) are emitted during chunk c+1
        pending_imm = []
        pending_tail = []

        n_ch = len(chunks)
        for ci, (c0, csz) in enumerate(chunks):
            if ci + PREFETCH < n_ch:
                xh_tiles[ci + PREFETCH] = issue_xh(ci + PREFETCH)
            xh_t = xh_tiles.pop(ci)
            x_sb = xh_t[:, 0:csz]
            h_sb = xh_t[:, csz : 2 * csz]

            p_r = psum.tile([H, csz], f32, tag="p_r")
            p_z = psum.tile([H, csz], f32, tag="p_z")
            p_in = psum.tile([H, csz], f32, tag="p_in")
            p_hn = psum.tile([H, csz], f32, tag="p_hn")

            # previous chunk's identity-matmul lands FIRST: it must
            # precede this chunk's p_in matmul in PE order (that matmul
            # WAR-waits on the previous tanh, which needs the I-matmul —
            # emitting it later would deadlock the PE queue)
            if pending_imm:
                pending_imm.pop(0)()

            # PE: weight-outer gate matmuls.  Group order r, z, hn, in
            # matches when each PSUM tag is freed by its consumer
            # (sig_r early, sig_z next, t mid, tanh of c-1 late).
            qs = [(q0, min(512, csz - q0)) for q0 in range(0, csz, 512)]
            for wcol, ptile, rhs, start, stop in (
                (0, p_r, x_sb, True, False),
                (1, p_r, h_sb, False, True),
                (2, p_z, x_sb, True, False),
                (3, p_z, h_sb, False, True),
                (5, p_hn, h_sb, True, True),
                (4, p_in, x_sb, True, False),   # group stays open for I @ t
            ):
                w_sl = wx0_sb[:, wcol * H : (wcol + 1) * H]
                for q0, qsz in qs:
                    qd = slice(q0, q0 + qsz)
                    nc.tensor.matmul(ptile[:, qd], w_sl, rhs[:, qd],
                                     start=start, stop=stop)

            # ACT: this chunk's sigmoids
            r_t = mids.tile([H, csz], bf16, tag="r")
            nc.scalar.activation(out=r_t, in_=p_r, func=sig,
                                 bias=b_sb[:, 0:1], scale=1.0)
            z_t = zs.tile([H, csz], bf16, tag="z")
            nc.scalar.activation(out=z_t, in_=p_z, func=sig,
                                 bias=b_sb[:, 1:2], scale=1.0)

            # previous chunk's tanh + blend + writeback (inputs all ready:
            # its I-matmul went to PE at the top of this chunk)
            if pending_tail:
                pending_tail.pop(0)()

            # DVE: t = (p_hn + b_hn) * r
            t_t = mids.tile([H, csz], bf16, tag="t")
            nc.vector.scalar_tensor_tensor(
                out=t_t, in0=p_hn, scalar=b_sb[:, 2:3], in1=r_t,
                op0=add_op, op1=mult_op)

            def imm(_p=p_in, _t=t_t, _csz=csz):
                for q0 in range(0, _csz, 512):
                    qsz = min(512, _csz - q0)
                    qd = slice(q0, q0 + qsz)
                    nc.tensor.matmul(_p[:, qd], eye_sb, _t[:, qd],
                                     start=False, stop=True)
            pending_imm.append(imm)

            def tail(_p=p_in, _z=z_t, _h=h_sb, _c0=c0, _csz=csz):
                nn = zs.tile([H, _csz], bf16, tag="n")
                nc.scalar.activation(out=nn, in_=_p, func=tanh,
                                     bias=b_sb[:, 3:4], scale=1.0)
                w_ch = wide.tile([H, _csz], bf16, tag="w")
                y_ch = wide.tile([H, _csz], bf16, tag="y")
                o_ch = wide.tile([H, _csz], bf16, tag="o")
                nc.vector.tensor_sub(out=w_ch, in0=_h, in1=nn)
                nc.vector.tensor_mul(out=y_ch, in0=_z, in1=w_ch)
                nc.vector.tensor_add(out=o_ch, in0=y_ch, in1=nn)
                nc.sync.dma_start(out=outT[:, _c0 : _c0 + _csz], in_=o_ch)
            pending_tail.append(tail)

        # drain the one-deep pipeline
        while pending_imm:
            pending_imm.pop(0)()
        while pending_tail:
            pending_tail.pop(0)()

    _patch_json(nc)
    return nc


def _get_nc(bpc: int) -> bass.Bass:
    if bpc not in _NC_CACHE:
        _NC_CACHE[bpc] = _build_nc(bpc)
    return _NC_CACHE[bpc]


def kernel(node_ids, messages, memory, W_ih, W_hh, b_ih, b_hh):
    global LAST_RESULT
    node_ids = np.asarray(node_ids)
    messages = np.asarray(messages, dtype=np.float32)
    memory = np.asarray(memory, dtype=np.float32)
    W_ih = np.asarray(W_ih, dtype=np.float32)
    W_hh = np.asarray(W_hh, dtype=np.float32)
    b_ih = np.asarray(b_ih, dtype=np.float32)
    b_hh = np.asarray(b_hh, dtype=np.float32)

    B = node_ids.shape[0]
    per = -(-B // N_CORES)                       # rows per core (unpadded)
    bpc = -(-per // 256) * 256                   # padded to 256 multiple
    nc = _get_nc(bpc)

    current = memory[node_ids]                   # [B, H] host gather

    bias = np.empty((H, 4), dtype=np.float32)
    bias[:, 0] = b_ih[0:H] + b_hh[0:H]
    bias[:, 1] = b_ih[H : 2 * H] + b_hh[H : 2 * H]
    bias[:, 2] = b_hh[2 * H : 3 * H]
    bias[:, 3] = b_ih[2 * H : 3 * H]

    # weight-outer column order: r_x, r_h, z_x, z_h, n_x, n_h, then the
    # identity that closes p_in's accumulation group
    wT = np.zeros((H, 7 * H), dtype=BF16)
    for g in range(3):
        wT[:, (2 * g) * H : (2 * g + 1) * H] = W_ih[g * H : (g + 1) * H].T
        wT[:, (2 * g + 1) * H : (2 * g + 2) * H] = W_hh[g * H : (g + 1) * H].T
    wT[:, 6 * H : 7 * H] = np.eye(H, dtype=BF16)

    in_maps = []
    for c in range(N_CORES):
        lo = c * per
        hi = min(lo + per, B)
        xh = np.zeros((H, 2, bpc), dtype=BF16)
        if hi > lo:
            xh[:, 0, : hi - lo] = messages[lo:hi].T
            xh[:, 1, : hi - lo] = current[lo:hi].T
        in_maps.append({"wT": wT, "xh": xh, "biases": bias})

    res = run_bass_kernel_spmd(nc, in_maps, list(range(N_CORES)))
    LAST_RESULT = res

    updated = np.empty((B, H), dtype=np.float32)
    for c in range(N_CORES):
        lo = c * per
        hi = min(lo + per, B)
        if hi > lo:
            updated[lo:hi] = res.results[c]["outT"][:, : hi - lo].T.astype(np.float32)

    new_memory = memory.copy()
    new_memory[node_ids] = updated
    return new_memory


# revision 6
# speedup vs baseline: 1.2899x; 1.0142x over previous
"""Trainium2 Bass kernel for the scatter_memory GRU memory-update module.

Computation (torch GRUCell semantics, chunk order r, z, n):
    current = memory[node_ids]                       # [B, H] gather
    gi = messages @ W_ih.T + b_ih ; gh = current @ W_hh.T + b_hh
    r = sigmoid(gi_r + gh_r) ; z = sigmoid(gi_z + gh_z)
    n = tanh(gi_n + r * gh_n)
    updated = (1 - z) * n + z * current
    new_memory = memory.at[node_ids].set(updated)    # scatter

Distribution: the B updated rows are sharded contiguously across 8
NeuronCores.  The gather/scatter over the 500k-row table and the
feature-major transposes run on the host; each core runs the GRU math on
its own [H, B/8] shard (feature dim H=128 sits on the SBUF partition
axis, so the GRU biases become per-partition vectors that fuse into the
ScalarEngine activation ops for free).

Engine layout (from several HW NTFF profiling rounds; at steady state
PE, ACT and DVE are all ~100% busy and the pace is ACT's
3 x (1024+352)/1.2ns per 1024-column chunk):
  PE   6 gate matmuls (bf16) + an identity-matmul accumulating
       t = r*(gh_n + b_hn) into p_in's still-open PSUM group, deferred
       one chunk so PE never waits on the ACT->DVE chain; ~9 dummy
       matmuls on zeroed tiles run during the DMA ramp so the PE clock
       (1.2 GHz for the first ~3.4us of activity, 2.4 GHz after) is
       warm when the first real chunk arrives
  ACT  sigmoid(r), sigmoid(z) per chunk + the previous chunk's tanh
       straight from PSUM
  DVE  t = (p_hn + b_hn)*r (the one PSUM-touching op) and the whole
       blend out = n + z*(h-n) as three bf16 SBUF TensorTensor ops in
       2x perf mode
  Pool UNUSED on purpose: concurrent GpSimd TensorTensor traffic slows
       DVE's 2x-mode ops 2-3.5x (SBUF port contention)
DMA order on the Sync HWDGE queue = first-need order: r/z gate weights,
first xh chunk, remaining weights, biases, then the chunk stream
interleaved with output writebacks.  (A previous revision queued five
big xh chunks ahead of the weights, pushing the first matmul to 12.5us.)
"""

import os
import sys

import numpy as np

for _p in ("/opt/trn_rl_repo", "/root/.axon_site/_ro/trn_rl_repo"):
    if os.path.isdir(_p) and _p not in sys.path:
        sys.path.insert(0, _p)

import ml_dtypes
from contextlib import ExitStack

import concourse.bass as bass
import concourse.tile as tile
from concourse import mybir
from concourse.bass_utils import run_bass_kernel_spmd

BF16 = ml_dtypes.bfloat16
import json as _json

N_CORES = 8
H = 128
NTILE = 1024

# exposed for test harnesses
LAST_RESULT = None

_NC_CACHE = {}


def _split_sync_waits(bir: dict) -> dict:
    """Hoist extra per-instruction semaphore waits into standalone
    EventSemaphore instructions.

    The walrus build in this container encodes at most ONE sync wait per
    instruction ("Too many sync wait commands" otherwise); Tile attaches
    one wait per dependency.  An engine-level standalone wait immediately
    before the instruction is semantically identical (the engine stalls
    either way), so keep the last wait inline and hoist the rest.
    """
    n = 0
    for fn in bir.get("functions", []):
        for blk in fn.get("blocks", []):
            out = []
            for inst in blk.get("instructions", []):
                si = inst.get("sync_info") or {}
                ow = si.get("on_wait") or []
                if len(ow) > 1:
                    for w in ow[:-1]:
                        n += 1
                        out.append({
                            "debug": inst.get("debug", 0),
                            "engine": inst["engine"],
                            "ins": [],
                            "outs": [],
                            "name": f"hoistw_{n}_{inst['name']}",
                            "opcode": "EventSemaphore",
                            "sync_info": {"on_update": [], "on_wait": [w]},
                        })
                    si["on_wait"] = [ow[-1]]
                out.append(inst)
            blk["instructions"] = out
    return bir


def _patch_json(nc: bass.Bass) -> None:
    orig = nc.to_json_bytes

    def patched() -> bytes:
        return _json.dumps(_split_sync_waits(_json.loads(orig()))).encode()

    nc.to_json_bytes = patched


def _chunk_plan(bpc: int) -> list[tuple[int, int]]:
    """512 head chunk (compute starts as soon as the small first DMA
    lands), 1024 middles, small tail so the final serial chain is
    short."""
    tail = [512, 256]
    mid = bpc - sum(tail)
    sizes = []
    while mid % NTILE:
        sizes.append(512)
        mid -= 512
    sizes += [NTILE] * (mid // NTILE)
    sizes += tail
    out = []
    pos = 0
    for c in sizes:
        out.append((pos, c))
        pos += c
    assert pos == bpc, (pos, bpc)
    return out


def _build_nc(bpc: int) -> bass.Bass:
    """Bass program for one core: GRU over a [H, bpc] feature-major shard."""
    assert bpc % 256 == 0
    f32 = mybir.dt.float32
    bf16 = mybir.dt.bfloat16
    sig = mybir.ActivationFunctionType.Sigmoid
    tanh = mybir.ActivationFunctionType.Tanh
    add_op = mybir.AluOpType.add
    mult_op = mybir.AluOpType.mult

    nc = bass.Bass()
    # wT column blocks: r_x, r_h, z_x, z_h, n_x, n_h, I (identity closes
    # the p_in accumulation group on PE); split into two DRAM tensors so
    # the first matmul only waits on the 2-gate-column transfer
    wTa = nc.declare_dram_parameter("wTa", [H, 2 * H], bf16, isOutput=False)
    wTb = nc.declare_dram_parameter("wTb", [H, 5 * H], bf16, isOutput=False)
    # xh packs x and h per partition so ONE DMA per chunk brings both
    xh = nc.declare_dram_parameter("xh", [H, 2, bpc], bf16, isOutput=False)
    # bias columns: 0 = b_ih_r + b_hh_r, 1 = b_ih_z + b_hh_z, 2 = b_hh_n, 3 = b_ih_n
    biases = nc.declare_dram_parameter("biases", [H, 4], f32, isOutput=False)
    outT = nc.declare_dram_parameter("outT", [H, bpc], bf16, isOutput=True)

    chunks = _chunk_plan(bpc)
    PREFETCH = 4  # xh chunk DMAs in flight ahead of compute

    with ExitStack() as ctx:
        tc = ctx.enter_context(tile.TileContext(nc))
        singles = ctx.enter_context(tc.tile_pool(name="singles", bufs=1))
        io = ctx.enter_context(tc.tile_pool(name="io", bufs=PREFETCH + 2))
        zs = ctx.enter_context(tc.tile_pool(name="zs", bufs=4))
        mids = ctx.enter_context(tc.tile_pool(name="mids", bufs=3))
        wide = ctx.enter_context(tc.tile_pool(name="wide", bufs=3))
        # 4 fixed [128, 1024] fp32 tags, 2 banks each -> all 8 banks
        psum = ctx.enter_context(tc.tile_pool(name="psum", bufs=1, space="PSUM"))

        # dummy sigmoid on a freshly-memset tile fires the ~1.3us ACT table
        # load immediately (zero DMA dependencies), so it overlaps the DMA
        # ramp instead of stalling the first real sigmoid
        warm_in = singles.tile([H, 1], f32)
        nc.vector.memset(warm_in, 0.0)
        warm_sb = singles.tile([H, 1], f32)
        nc.scalar.activation(out=warm_sb, in_=warm_in,
                             func=sig, bias=0.0, scale=1.0)

        # Sync HWDGE queue in first-need order: r/z-gate weights, the
        # first xh chunk, the remaining weights, biases
        wa_sb = singles.tile([H, 2 * H], bf16)
        nc.sync.dma_start(out=wa_sb, in_=wTa[:, :])

        def issue_xh(ci: int):
            c0, csz = chunks[ci]
            t = io.tile([H, 2 * csz], bf16, tag="xh")
            nc.sync.dma_start(out=t, in_=xh[:, :, c0 : c0 + csz])
            return t

        xh_tiles: dict[int, object] = {0: issue_xh(0)}
        wb_sb = singles.tile([H, 5 * H], bf16)
        nc.sync.dma_start(out=wb_sb, in_=wTb[:, :])
        b_sb = singles.tile([H, 4], f32)
        nc.sync.dma_start(out=b_sb, in_=biases[:, :])
        for ci in range(1, min(PREFETCH, len(chunks))):
            xh_tiles[ci] = issue_xh(ci)

        eye_sb = wb_sb[:, 4 * H : 5 * H]

        def wsl(wcol: int):
            if wcol < 2:
                return wa_sb[:, wcol * H : (wcol + 1) * H]
            return wb_sb[:, (wcol - 2) * H : (wcol - 1) * H]

        # PE clock warmup: ~4us of dummy matmuls (zeroed operands, no DMA
        # dependencies) run while the weight/xh DMAs are in flight, so
        # the HAM activity window reaches the 2.4 GHz p-state before the
        # first real matmul.  They write the p_r PSUM banks, which chunk
        # 0 then WAW-reuses.
        wz_sb = singles.tile([H, H], bf16)
        nc.vector.memset(wz_sb, 0.0)
        xz_sb = singles.tile([H, 512], bf16)
        nc.vector.memset(xz_sb, 0.0)
        p_warm = psum.tile([H, NTILE], f32, tag="p_r")
        for _ in range(9):
            nc.tensor.matmul(p_warm[:, 0:512], wz_sb, xz_sb,
                             start=True, stop=True)

        # one-chunk software pipeline: chunk c's identity-matmul (closing
        # p_in's open group with + I @ t) and everything downstream of it
        # (tanh, blend, writeback) are emitted during chunk c+1
        pending_imm = []
        pending_tail = []

        n_ch = len(chunks)
        for ci, (c0, csz) in enumerate(chunks):
            if ci + PREFETCH < n_ch:
                xh_tiles[ci + PREFETCH] = issue_xh(ci + PREFETCH)
            xh_t = xh_tiles.pop(ci)
            x_sb = xh_t[:, 0:csz]
            h_sb = xh_t[:, csz : 2 * csz]

            # fixed-size PSUM tags so the pool footprint is exactly 8
            # banks regardless of the chunk-size taper
            p_r_full = psum.tile([H, NTILE], f32, tag="p_r")
            p_z_full = psum.tile([H, NTILE], f32, tag="p_z")
            p_in_full = psum.tile([H, NTILE], f32, tag="p_in")
            p_hn_full = psum.tile([H, NTILE], f32, tag="p_hn")
            p_r, p_z = p_r_full[:, 0:csz], p_z_full[:, 0:csz]
            p_in, p_hn = p_in_full[:, 0:csz], p_hn_full[:, 0:csz]

            # previous chunk's identity-matmul lands FIRST: it must
            # precede this chunk's p_in matmul in PE order (that matmul
            # WAR-waits on the previous tanh, which needs the I-matmul —
            # emitting it later would deadlock the PE queue)
            if pending_imm:
                pending_imm.pop(0)()

            # PE: weight-outer gate matmuls.  Group order r, z, hn, in
            # matches when each PSUM tag is freed by its consumer
            # (sig_r early, sig_z next, t mid, tanh of c-1 late).
            qs = [(q0, min(512, csz - q0)) for q0 in range(0, csz, 512)]
            for wcol, ptile, rhs, start, stop in (
                (0, p_r, x_sb, True, False),
                (1, p_r, h_sb, False, True),
                (2, p_z, x_sb, True, False),
                (3, p_z, h_sb, False, True),
                (5, p_hn, h_sb, True, True),
                (4, p_in, x_sb, True, False),   # group stays open for I @ t
            ):
                w_sl = wsl(wcol)
                for q0, qsz in qs:
                    qd = slice(q0, q0 + qsz)
                    nc.tensor.matmul(ptile[:, qd], w_sl, rhs[:, qd],
                                     start=start, stop=stop)

            # ACT: this chunk's sigmoids
            r_t = mids.tile([H, csz], bf16, tag="r")
            nc.scalar.activation(out=r_t, in_=p_r, func=sig,
                                 bias=b_sb[:, 0:1], scale=1.0)
            z_t = zs.tile([H, csz], bf16, tag="z")
            nc.scalar.activation(out=z_t, in_=p_z, func=sig,
                                 bias=b_sb[:, 1:2], scale=1.0)

            # previous chunk's tanh + blend + writeback (inputs all ready:
            # its I-matmul went to PE at the top of this chunk)
            if pending_tail:
                pending_tail.pop(0)()

            # DVE: t = (p_hn + b_hn) * r
            t_t = mids.tile([H, csz], bf16, tag="t")
            nc.vector.scalar_tensor_tensor(
                out=t_t, in0=p_hn, scalar=b_sb[:, 2:3], in1=r_t,
                op0=add_op, op1=mult_op)

            def imm(_p=p_in, _t=t_t, _csz=csz):
                for q0 in range(0, _csz, 512):
                    qsz = min(512, _csz - q0)
                    qd = slice(q0, q0 + qsz)
                    nc.tensor.matmul(_p[:, qd], eye_sb, _t[:, qd],
                                     start=False, stop=True)
            pending_imm.append(imm)

            def tail(_p=p_in, _z=z_t, _h=h_sb, _c0=c0, _csz=csz):
                nn = zs.tile([H, _csz], bf16, tag="n")
                nc.scalar.activation(out=nn, in_=_p, func=tanh,
                                     bias=b_sb[:, 3:4], scale=1.0)
                w_ch = wide.tile([H, _csz], bf16, tag="w")
                y_ch = wide.tile([H, _csz], bf16, tag="y")
                o_ch = wide.tile([H, _csz], bf16, tag="o")
                nc.vector.tensor_sub(out=w_ch, in0=_h, in1=nn)
                nc.vector.tensor_mul(out=y_ch, in0=_z, in1=w_ch)
                nc.vector.tensor_add(out=o_ch, in0=y_ch, in1=nn)
                nc.sync.dma_start(out=outT[:, _c0 : _c0 + _csz], in_=o_ch)
            pending_tail.append(tail)

        # drain the one-deep pipeline
        while pending_imm:
            pending_imm.pop(0)()
        while pending_tail:
            pending_tail.pop(0)()

    _patch_json(nc)
    return nc


def _get_nc(bpc: int) -> bass.Bass:
    if bpc not in _NC_CACHE:
        _NC_CACHE[bpc] = _build_nc(bpc)
    return _NC_CACHE[bpc]


def kernel(node_ids, messages, memory, W_ih, W_hh, b_ih, b_hh):
    global LAST_RESULT
    node_ids = np.asarray(node_ids)
    messages = np.asarray(messages, dtype=np.float32)
    memory = np.asarray(memory, dtype=np.float32)
    W_ih = np.asarray(W_ih, dtype=np.float32)
    W_hh = np.asarray(W_hh, dtype=np.float32)
    b_ih = np.asarray(b_ih, dtype=np.float32)
    b_hh = np.asarray(b_hh, dtype=np.float32)

    B = node_ids.shape[0]
    per = -(-B // N_CORES)                       # rows per core (unpadded)
    bpc = -(-per // 256) * 256                   # padded to 256 multiple
    nc = _get_nc(bpc)

    current = memory[node_ids]                   # [B, H] host gather

    bias = np.empty((H, 4), dtype=np.float32)
    bias[:, 0] = b_ih[0:H] + b_hh[0:H]
    bias[:, 1] = b_ih[H : 2 * H] + b_hh[H : 2 * H]
    bias[:, 2] = b_hh[2 * H : 3 * H]
    bias[:, 3] = b_ih[2 * H : 3 * H]

    # weight-outer column order: r_x, r_h | z_x, z_h, n_x, n_h, I
    wTa = np.empty((H, 2 * H), dtype=BF16)
    wTa[:, 0:H] = W_ih[0:H].T
    wTa[:, H : 2 * H] = W_hh[0:H].T
    wTb = np.zeros((H, 5 * H), dtype=BF16)
    for g in (1, 2):
        wTb[:, (2 * g - 2) * H : (2 * g - 1) * H] = W_ih[g * H : (g + 1) * H].T
        wTb[:, (2 * g - 1) * H : (2 * g) * H] = W_hh[g * H : (g + 1) * H].T
    wTb[:, 4 * H : 5 * H] = np.eye(H, dtype=BF16)

    in_maps = []
    for c in range(N_CORES):
        lo = c * per
        hi = min(lo + per, B)
        xh = np.zeros((H, 2, bpc), dtype=BF16)
        if hi > lo:
            xh[:, 0, : hi - lo] = messages[lo:hi].T
            xh[:, 1, : hi - lo] = current[lo:hi].T
        in_maps.append({"wTa": wTa, "wTb": wTb, "xh": xh, "biases": bias})

    res = run_bass_kernel_spmd(nc, in_maps, list(range(N_CORES)))
    LAST_RESULT = res

    updated = np.empty((B, H), dtype=np.float32)
    for c in range(N_CORES):
        lo = c * per
        hi = min(lo + per, B)
        if hi > lo:
            updated[lo:hi] = res.results[c]["outT"][:, : hi - lo].T.astype(np.float32)

    new_memory = memory.copy()
    new_memory[node_ids] = updated
    return new_memory
